# revision 16
# baseline (speedup 1.0000x reference)
"""Trainium2 Bass kernel for nn_AttentionLayer (B=2, S=2048, HID=1024, H=16, D=64).

Sharding: 8 cores = 2 (batch) x 4 (head-groups of 4 heads).
Each core computes q/k/v projections for its 4 heads, rotary (degenerate
elementwise multiply), scores^T, softmax (no max-subtraction; scores bounded
~+-8), multiplicative attention bias, probs @ v, and a partial output
projection with its slice of Wo rows. Host sums the 4 partials per batch.

v2 (PE-column minimization): the cost model and measured trace show PE time
= total streamed output columns x ~0.37ns, and the baseline was 100% PE-bound
at 524288 cols. Changes vs baseline:
- softmax denominator: e-tiles pair-summed on DVE (s2 = e_2i + e_2i+1, fused
  [128,2,1024] adds), so the ones-matmul Z streams 8 instead of 16 tiles per
  pair: Z cols 131072 -> 65536.
- e/bias/v/ebn path in fp16 (same PE rate, better precision than bf16).
- ebn = e*bias as one fused [128,2,1024] DVE mul per 2 slots with a stride-0
  broadcast view of the bias tile (heads share bias).
- v-path PSUM->SBUF casts moved to the ACT engine (idle outside exp).
- q/k/v/Wo projection groups spread across all 8 attention pairs instead of
  cramming into phase A, so light slots stay above the ACT exp rate.
- fp16 output partials (host sums in fp32): halves output DMA.
"""

import math
import os
import sys

import numpy as np

for _p in ("/opt/trn_rl_repo", "/root/.axon_site/_ro/trn_rl_repo"):
    if os.path.isdir(_p) and _p not in sys.path:
        sys.path.append(_p)

import ml_dtypes  # noqa: E402

import concourse.bass as bass  # noqa: E402
import concourse.bacc as bacc  # noqa: E402
import concourse.mybir as mybir  # noqa: E402
import concourse.tile as tile  # noqa: E402
from concourse.bass import ts  # noqa: E402
from concourse.bass_utils import run_bass_kernel_spmd  # noqa: E402

B, S, HID = 2, 2048, 1024
D = 64
H = 16
ROT = 32
NCORES = 8
GH = 4            # heads per core
DG = GH * D       # 256 d-columns per core
NSJ = 16          # sj tiles of 128
NSC = 4           # si chunks
SC = S // NSC     # 512 si per chunk
NKT = HID // 128  # 8 contraction tiles for projections
NST = S // 128    # 16 s tiles
LAG = 5

F32 = mybir.dt.float32
PDT = mybir.dt.float16         # q/k/scores and o/Wo path
EDT = mybir.dt.bfloat16        # e/bias/v path (scores reach +-32: exp needs
NP_PDT = np.float16            # bf16 range; softmax cancels the scale)
NP_EDT = ml_dtypes.bfloat16

_PROGRAM = None


def _install_neff_cache():
    """Cache BIR->NEFF compiles on disk."""
    import hashlib
    import shutil

    import concourse.bass_utils as _bu
    import concourse.bass2jax as _b2j

    if getattr(_bu.compile_bir_kernel, "_neff_cached", False):
        return
    cache_dir = os.environ.get(
        "BASS_NEFF_CACHE", os.path.expanduser("~/.bass_neff_cache")
    )
    os.makedirs(cache_dir, exist_ok=True)
    orig = _bu.compile_bir_kernel

    def cached(bir_json, tmpdir, neff_name="file.neff"):
        salt = os.environ.get("BASS_LDW_OPT", "0").encode()
        key = hashlib.sha256(bir_json + salt).hexdigest()
        hit = os.path.join(cache_dir, key + ".neff")
        dst = os.path.join(tmpdir, neff_name)
        if os.path.exists(hit):
            shutil.copy(hit, dst)
            return dst
        path = orig(bir_json, tmpdir, neff_name)
        try:
            shutil.copy(path, hit)
        except OSError:
            pass
        return path

    cached._neff_cached = True
    _bu.compile_bir_kernel = cached
    _b2j.compile_bir_kernel = cached

    if os.environ.get("BASS_LDW_OPT", "0") == "1":
        orig_rc = _bu.run_command

        def run_command_ldw(argv, **kwargs):
            argv = ["--enable-ldw-opt=true" if a == "--enable-ldw-opt=false"
                    else a for a in argv]
            return orig_rc(argv, **kwargs)

        _bu.run_command = run_command_ldw


_install_neff_cache()


def build_kernel_body(tc):
    nc = tc.nc
    Exp = mybir.ActivationFunctionType.Exp

    xTd = nc.dram_tensor("xT", [128, NSC, NKT, SC], PDT, kind="ExternalInput").ap()
    wqd = nc.dram_tensor("wq", [128, NKT, DG], PDT, kind="ExternalInput").ap()
    wkd = nc.dram_tensor("wk", [128, NKT, DG], PDT, kind="ExternalInput").ap()
    wvd = nc.dram_tensor("wv", [128, NKT, DG], EDT, kind="ExternalInput").ap()
    wod = nc.dram_tensor("wo", [128, 2, HID], PDT, kind="ExternalInput").ap()
    rope = nc.dram_tensor("rope", [128, S], F32, kind="ExternalInput").ap()
    biasd = nc.dram_tensor("biasT", [NSC, 128, NSJ, SC], EDT, kind="ExternalInput").ap()
    onesd = nc.dram_tensor("onesd", [128, 128], EDT, kind="ExternalInput").ap()
    out = nc.dram_tensor("out", [S, HID], PDT, kind="ExternalOutput").ap()

    import contextlib
    ctx = contextlib.ExitStack()
    with ctx:
        keep = ctx.enter_context(tc.tile_pool(name="keep", bufs=1))
        pa = ctx.enter_context(tc.tile_pool(name="phaseA", bufs=1))
        e_pool = ctx.enter_context(tc.tile_pool(name="ep", bufs=4))
        s2_pool = ctx.enter_context(tc.tile_pool(name="s2p", bufs=2))
        ebn_pool = ctx.enter_context(tc.tile_pool(name="ebnp", bufs=4))
        bias_pool = ctx.enter_context(tc.tile_pool(name="biasp", bufs=2))
        o_pool = ctx.enter_context(tc.tile_pool(name="op", bufs=2))
        rz_pool = ctx.enter_context(tc.tile_pool(name="rzp", bufs=2))
        oout_pool = ctx.enter_context(tc.tile_pool(name="oout", bufs=4))

        pp_proj = ctx.enter_context(tc.tile_pool(name="pp_proj", bufs=2, space="PSUM"))
        pp_s = ctx.enter_context(tc.tile_pool(name="pp_s", bufs=2, space="PSUM"))
        pp_z = ctx.enter_context(tc.tile_pool(name="pp_z", bufs=1, space="PSUM"))
        pp_o = ctx.enter_context(tc.tile_pool(name="pp_o", bufs=1, space="PSUM"))

        # ---- DMA loads (order = trigger order on the Sync queue) ----
        wq_s = pa.tile([128, NKT, DG], PDT)
        wk_s = pa.tile([128, NKT, DG], PDT)
        wv_s = pa.tile([128, NKT, DG], EDT)
        rope_s = pa.tile([128, S], F32)
        xts = pa.tile([128, NSC, NKT, SC], PDT)
        nc.sync.dma_start(out=wk_s[:, 0:2], in_=wkd[:, 0:2])
        nc.sync.dma_start(out=xts[:, 0, 0:2], in_=xTd[:, 0, 0:2])
        nc.sync.dma_start(out=wk_s[:, 2:4], in_=wkd[:, 2:4])
        nc.sync.dma_start(out=xts[:, 0, 2:4], in_=xTd[:, 0, 2:4])
        nc.sync.dma_start(out=wk_s[:, 4:8], in_=wkd[:, 4:8])
        nc.sync.dma_start(out=xts[:, 0, 4:8], in_=xTd[:, 0, 4:8])
        nc.sync.dma_start(out=wq_s[:, 0:4], in_=wqd[:, 0:4])
        nc.sync.dma_start(out=wq_s[:, 4:8], in_=wqd[:, 4:8])
        nc.sync.dma_start(out=rope_s[:, 0:SC], in_=rope[:, 0:SC])
        nc.sync.dma_start(out=wv_s[:], in_=wvd[:])
        nc.sync.dma_start(out=xts[:, 1], in_=xTd[:, 1])

        bias_tiles = {}

        def load_bias(c):
            bias_c = bias_pool.tile([128, NSJ, SC], EDT, tag="bias", name="biasc")
            nc.sync.dma_start(out=bias_c[:], in_=biasd[c])
            bias_tiles[c] = bias_c

        load_bias(0)
        nc.sync.dma_start(out=rope_s[:, SC:], in_=rope[:, SC:])
        nc.sync.dma_start(out=xts[:, 2], in_=xTd[:, 2])
        nc.sync.dma_start(out=xts[:, 3], in_=xTd[:, 3])
        wo_s = keep.tile([128, 2, HID], PDT)
        nc.sync.dma_start(out=wo_s[:], in_=wod[:])
        ones_s = keep.tile([128, 128], EDT)
        nc.sync.dma_start(out=ones_s[:], in_=onesd[:])
        load_bias(1)

        kt_s = keep.tile([128, 2, S], PDT)
        qt_s = keep.tile([128, 2, S], PDT)
        v_s = keep.tile([128, NST, DG], EDT)

        # ---- projection building blocks ----
        def proj_group(w_s, slab, dt, sc):
            ps = pp_proj.tile([128, 512], F32, tag="ppp", name="ps")
            for kt in range(NKT):
                nc.tensor.matmul(
                    ps[:], lhsT=w_s[:, kt, ts(dt, 128)],
                    rhs=xts[:, sc, kt, :],
                    start=(kt == 0), stop=(kt == NKT - 1),
                )
            nc.vector.tensor_mul(
                slab[:, dt, ts(sc, SC)], ps[:], rope_s[:, ts(sc, SC)])

        def v_group(st):
            ps = pp_proj.tile([128, 512], F32, tag="ppp", name="ps")
            for kt in range(NKT):
                nc.tensor.matmul(
                    ps[:, 0:DG], lhsT=xts[:, st // 4, kt, ts(st % 4, 128)],
                    rhs=wv_s[:, kt, :],
                    start=(kt == 0), stop=(kt == NKT - 1),
                )
            nc.scalar.copy(v_s[:, st, :], ps[:, 0:DG])

        # ---- attention state ----
        class PairState:
            pass

        def new_pair(c, pair):
            st_ = PairState()
            st_.c, st_.pair = c, pair
            st_.zps = pp_z.tile([128, SC], F32, tag="z", name="zps")
            st_.ops = pp_o.tile([128, SC], F32, tag="o", name="ops")
            st_.sq = {}
            st_.eblk = [None] * 4
            st_.s2 = [None] * 4
            st_.ebn = [None] * 8
            return st_

        def do_scores_exp(st_, j):
            c, pair = st_.c, st_.pair
            b, r = divmod(j, 4)
            if r == 0:
                st_.eblk[b] = e_pool.tile([128, 4, 2 * SC], EDT, tag="e",
                                          name="eblk")
            sq = pp_s.tile([128, 2 * SC], F32, tag="s", name="sq")
            nc.tensor.matmul(
                sq[:, 0:SC], lhsT=kt_s[0:64, pair, ts(j, 128)],
                rhs=qt_s[0:64, pair, ts(c, SC)],
                start=True, stop=True, tile_position=(0, 0),
                skip_group_check=True,
            )
            nc.tensor.matmul(
                sq[:, SC:2 * SC], lhsT=kt_s[64:128, pair, ts(j, 128)],
                rhs=qt_s[64:128, pair, ts(c, SC)],
                start=True, stop=True, tile_position=(64, 0),
                skip_group_check=True,
            )
            nc.scalar.activation(st_.eblk[b][:, r, :], sq[:], Exp)

        def do_ebn(st_, u, bias_c):
            # ebn for sj-tiles 2u, 2u+1: one fused [128,2,1024] mul with a
            # stride-0 broadcast of the shared-bias tile over the 2 heads.
            b, hh = divmod(u, 2)
            ebn = ebn_pool.tile([128, 2, 2 * SC], EDT, tag="ebn", name="ebn")
            st_.ebn[u] = ebn
            e4 = st_.eblk[b][:, 2 * hh:2 * hh + 2, :].rearrange(
                "p a (h f) -> p a h f", f=SC)
            b4 = bias_c[:, 2 * u:2 * u + 2, :].unsqueeze(2).broadcast_to(
                [128, 2, 2, SC])
            o4 = ebn[:].rearrange("p a (h f) -> p a h f", f=SC)
            nc.vector.tensor_mul(o4, e4, b4)

        def do_l1(st_, b):
            # s2[2b+a] = e[4b+2a] + e[4b+2a+1], a in {0,1}: one fused add
            s2 = s2_pool.tile([128, 2, 2 * SC], EDT, tag="s2", name="s2")
            st_.s2[b] = s2
            er = st_.eblk[b][:].rearrange("p (a two) f -> p a two f", two=2)
            nc.vector.tensor_add(s2[:], er[:, :, 0, :], er[:, :, 1, :])

        def do_z(st_, b):
            s2 = st_.s2[b]
            for a in range(2):
                first = b == 0 and a == 0
                last = b == 3 and a == 1
                nc.tensor.matmul(
                    st_.zps[0:64, :], lhsT=ones_s[:, 0:64],
                    rhs=s2[:, a, 0:SC],
                    start=first, stop=last, tile_position=(0, 0),
                    skip_group_check=True,
                )
                nc.tensor.matmul(
                    st_.zps[64:128, :], lhsT=ones_s[:, 0:64],
                    rhs=s2[:, a, SC:2 * SC],
                    start=first, stop=last, tile_position=(0, 64),
                    skip_group_check=True,
                )

        def do_pv(st_, j):
            pair = st_.pair
            ebn = st_.ebn[j // 2][:, j % 2, :]
            first, last = j == 0, j == NSJ - 1
            nc.tensor.matmul(
                st_.ops[0:64, :], lhsT=v_s[:, j, ts(2 * pair, 64)],
                rhs=ebn[:, 0:SC],
                start=first, stop=last, tile_position=(0, 0),
                skip_group_check=True,
            )
            nc.tensor.matmul(
                st_.ops[64:128, :], lhsT=v_s[:, j, ts(2 * pair + 1, 64)],
                rhs=ebn[:, SC:2 * SC],
                start=first, stop=last, tile_position=(0, 64),
                skip_group_check=True,
            )

        def finalize_pair(st_):
            rz = rz_pool.tile([128, SC], F32, tag="rz", name="rz")
            nc.vector.reciprocal_approx_fast(out=rz[:], in_=st_.zps[:])
            o_t = o_pool.tile([128, SC], PDT, tag=f"o{st_.pair}", name="ot")
            nc.vector.tensor_mul(o_t[:], st_.ops[:], rz[:])
            return o_t

        def wo_group(o_tiles, c, stl, hc):
            wps = pp_proj.tile([128, 512], F32, tag="ppp", name="wps")
            for pair in range(2):
                nc.tensor.matmul(
                    wps[:], lhsT=o_tiles[pair][:, ts(stl, 128)],
                    rhs=wo_s[:, pair, ts(hc, 512)],
                    start=(pair == 0), stop=(pair == 1),
                )
            oo = oout_pool.tile([128, 512], PDT, tag="oo", name="oo")
            nc.vector.tensor_copy(oo[:], wps[:])
            nc.sync.dma_start(out=out[ts(c * 4 + stl, 128), ts(hc, 512)],
                              in_=oo[:])

        # ---- fillers: (pair_index, slot) -> list of thunks ----
        # pair (c,p) scores need kt slab p (all chunks) + qt slab p chunk c.
        fillers = {}

        def add_filler(pi, j, fn):
            fillers.setdefault((pi, j), []).append(fn)

        # phase A (pi=0, pair (0,0)): v groups every slot; k(0,sc) just in
        # time for sj-tile 4sc; k/q slabs needed by (0,1) late in the pair.
        for j in range(NSJ):
            add_filler(0, j, lambda st=j: v_group(st))
        add_filler(0, 3, lambda: proj_group(wk_s, kt_s, 0, 1))
        add_filler(0, 5, lambda: proj_group(wk_s, kt_s, 0, 2))
        add_filler(0, 7, lambda: proj_group(wk_s, kt_s, 0, 3))
        add_filler(0, 9, lambda: proj_group(wq_s, qt_s, 1, 0))
        add_filler(0, 11, lambda: proj_group(wk_s, kt_s, 1, 0))
        # pi=1 (0,1): rest of k slab 1; q(0,1) for (1,0)
        add_filler(1, 1, lambda: proj_group(wk_s, kt_s, 1, 1))
        add_filler(1, 5, lambda: proj_group(wk_s, kt_s, 1, 2))
        add_filler(1, 7, lambda: proj_group(wk_s, kt_s, 1, 3))
        add_filler(1, 11, lambda: proj_group(wq_s, qt_s, 0, 1))
        # q slab for pair (c,1) prepared during (c,0); q(0,c+1) during (c,1)
        add_filler(2, 11, lambda: proj_group(wq_s, qt_s, 1, 1))
        add_filler(3, 11, lambda: proj_group(wq_s, qt_s, 0, 2))
        add_filler(4, 11, lambda: proj_group(wq_s, qt_s, 1, 2))
        add_filler(5, 11, lambda: proj_group(wq_s, qt_s, 0, 3))
        add_filler(6, 11, lambda: proj_group(wq_s, qt_s, 1, 3))

        # ---- lead-in ----
        proj_group(wk_s, kt_s, 0, 0)
        proj_group(wq_s, qt_s, 0, 0)

        # ---- pair chain with bridged transitions ----
        pair_seq = [(c, p) for c in range(NSC) for p in range(2)]
        o_done = {}

        def run_slot(st_, pi, j, wo_list):
            do_scores_exp(st_, j)
            if j >= 2 and j % 2 == 0:
                do_ebn(st_, (j - 2) // 2, bias_tiles[st_.c])
            if j >= LAG:
                do_pv(st_, j - LAG)
            if j % 4 == 0 and j >= 4:
                do_l1(st_, j // 4 - 1)
            for fn in fillers.get((pi, j), ()):
                fn()
            if wo_list and j >= 6 and j % 2 == 0:
                idx = (j - 6) // 2
                if idx < len(wo_list):
                    wo_list[idx]()
            if j >= 6 and (j - 6) % 4 == 0:
                do_z(st_, (j - 6) // 4)
            if j == 5 and pi % 2 == 0 and st_.c + 1 < NSC and pi >= 2:
                load_bias(st_.c + 1)

        def bridge(prev, cur, pi_cur):
            # drain prev tails interleaved with cur's first LAG scores
            for g in range(LAG):
                do_scores_exp(cur, g)
                do_pv(prev, NSJ - LAG + g)
                if g == 0:
                    do_ebn(prev, 7, bias_tiles[prev.c])
                if g == 1:
                    do_l1(prev, 3)
                if g == 3:
                    do_z(prev, 3)
                if g >= 2 and g % 2 == 0:
                    do_ebn(cur, (g - 2) // 2, bias_tiles[cur.c])
                if g == 4:
                    do_l1(cur, 0)
                for fn in fillers.get((pi_cur, g), ()):
                    fn()
            o_done[(prev.c, prev.pair)] = finalize_pair(prev)

        st = new_pair(0, 0)
        for j in range(NSJ):
            run_slot(st, 0, j, None)
        prev = st
        for pi, (c, p) in enumerate(pair_seq[1:], start=1):
            cur = new_pair(c, p)
            bridge(prev, cur, pi)
            # Wo groups for chunk c-1 run during (c,0) [4] and (c,1) [4];
            # built after bridge() so o_done[(c-1, p_prev)] exists.
            wo_list = []
            if c >= 1:
                oc = c - 1
                o_pc = [o_done[(oc, 0)], o_done[(oc, 1)]]
                units = [(stl, hc) for stl in range(4) for hc in range(2)]
                half = units[:4] if p == 0 else units[4:]
                wo_list = [
                    (lambda stl=stl, hc=hc, o_pc=o_pc, oc=oc:
                     wo_group(o_pc, oc, stl, hc))
                    for (stl, hc) in half
                ]
            for j in range(LAG, NSJ):
                run_slot(cur, pi, j, wo_list)
            prev = cur

        # ---- final drain ----
        for g in range(LAG):
            do_pv(prev, NSJ - LAG + g)
            if g == 0:
                do_ebn(prev, 7, bias_tiles[prev.c])
            if g == 1:
                do_l1(prev, 3)
            if g == 3:
                do_z(prev, 3)
        o_done[(prev.c, prev.pair)] = finalize_pair(prev)
        o_last = [o_done[(NSC - 1, 0)], o_done[(NSC - 1, 1)]]
        for stl in range(4):
            for hc in range(2):
                wo_group(o_last, NSC - 1, stl, hc)


def build_program():
    global _PROGRAM
    if _PROGRAM is not None:
        return _PROGRAM
    nc = bacc.Bacc(trn_type="TRN2", target_bir_lowering=False, debug=False,
                   num_devices=NCORES)
    with tile.TileContext(nc) as tc:
        build_kernel_body(tc)
    nc.compile()
    _PROGRAM = nc
    return nc


def make_in_maps(x, sinusoids, attention_bias, Wq, bq, Wk, bk, Wv, bv, Wo):
    assert not np.any(bq) and not np.any(bk) and not np.any(bv), (
        "kernel assumes zero q/k/v biases (reference setup uses zeros)"
    )
    x = np.asarray(x, np.float32)
    sinusoids = np.asarray(sinusoids, np.float32)
    attention_bias = np.asarray(attention_bias, np.float32)
    Wq = np.asarray(Wq, np.float32)
    Wk = np.asarray(Wk, np.float32)
    Wv = np.asarray(Wv, np.float32)
    Wo = np.asarray(Wo, np.float32)

    sgn = np.array([-1.0, 1.0] * (ROT // 2), np.float32)
    ones128 = np.ones((128, 128), NP_EDT)
    scale = np.float32(1.0 / math.sqrt(D))

    in_maps = []
    for core in range(NCORES):
        b, g = divmod(core, 4)
        sin_b = sinusoids[b, 0]
        cos_b = sinusoids[b, 1]
        mult = cos_b + sgn[None, :] * sin_b          # [S, ROT]
        rope = np.ones((128, S), np.float32)
        rope[0:32, :] = mult.T
        rope[64:96, :] = mult.T
        xTb = x[b].T.astype(NP_PDT)                      # [HID, S]
        xp = np.ascontiguousarray(
            xTb.reshape(NKT, 128, NSC, SC).transpose(1, 2, 0, 3))

        def packw(w, dt=NP_PDT):
            return np.ascontiguousarray(
                w.astype(dt).reshape(-1, 128, w.shape[1]).transpose(1, 0, 2))

        def packw_e(w):
            return packw(w, NP_EDT)
        ab = attention_bias[b, 0].astype(NP_EDT)         # [si, sj]
        biasp = np.ascontiguousarray(
            ab.reshape(NSC, SC, NSJ, 128).transpose(0, 3, 2, 1))
        in_maps.append({
            "xT": xp,
            "wq": packw(Wq[:, ts_np(g)] * scale),
            "wk": packw(Wk[:, ts_np(g)]),
            "wv": packw_e(Wv[:, ts_np(g)]),
            "wo": packw(Wo[ts_np(g), :]),
            "rope": rope,
            "biasT": biasp,
            "onesd": ones128,
        })
    return in_maps


def ts_np(g):
    return slice(g * DG, (g + 1) * DG)


def kernel(**inputs):
    nc = build_program()
    in_maps = make_in_maps(**inputs)
    res = run_bass_kernel_spmd(nc, in_maps, list(range(NCORES)))
    outs = res.results
    full = np.zeros((B, S, HID), np.float32)
    for core in range(NCORES):
        b = core // 4
        full[b] += np.asarray(outs[core]["out"], dtype=np.float32)
    return full
